# revision 10
# baseline (speedup 1.0000x reference)
"""Trainium2 Bass kernel for int4-grouped-quantized linear (GPTQ-style).

out[8192, 11008] = x[8192, 4096] @ dequant(qweight, qzeros, scales)

Sharding: column-parallel over out_features N across 8 NeuronCores.

Device-side structure per core:
  - Dequantize the W shard [4096, 1376] on-chip: qweight is viewed as bytes
    (host-side reinterpret), so each nibble extraction is a single int ALU op
    and fuses with the scale multiply via scalar_tensor_tensor:
        w*s = (qw_u8 & 0xF) * s_bc   /   (qw_u8 >> 4) * s_bc
    then one tensor_tensor subtract of the precomputed broadcast zs = z*s.
    This orders W's columns [all lo-nibbles | all hi-nibbles]; the host
    un-permutes output columns (pure reshape/transpose).
  - Dense fp16 matmuls on the PE with fp32 PSUM accumulation; x is
    pre-transposed/tiled on the host (layout only) so the stationary
    [128 k, 128 t] tiles stream straight from DRAM, no on-device transposes.
"""

import sys

sys.path.insert(0, "/opt/trn_rl_repo")

from contextlib import ExitStack

import numpy as np

import concourse.bass as bass
from concourse import bacc
import concourse.tile as tile
from concourse import mybir
from concourse.bass_utils import run_bass_kernel_spmd

AOT = mybir.AluOpType
F16, I32, U8 = mybir.dt.float16, mybir.dt.int32, mybir.dt.uint8
F32 = mybir.dt.float32

T, K, N = 8192, 4096, 11008
NCORES = 8
NS = N // NCORES  # 1376 out cols per core
CS = NS // 8  # 172 packed int32 cols per core
CB = CS * 4  # 688 packed bytes per core (= NS/2)
G = 32  # quant groups (group size 128 == one k-block)
KB = K // 128  # 32 k-blocks
QCH = 8  # k-blocks per qweight load chunk
TC = 512  # t columns per x.T chunk
NCH = T // TC  # 16 chunks
TBLK = TC // 128  # 4 output row-blocks per chunk
SEGS = [(0, 512), (512, 512), (1024, 352)]  # N segments (PSUM bank sized)


def _body(ctx, tc, xtd, qwd, comb, outd):
    nc = tc.nc
    qpool = ctx.enter_context(tc.tile_pool(name="qwp", bufs=2))
    stpool = ctx.enter_context(tc.tile_pool(name="stage", bufs=2))
    ftpool = ctx.enter_context(tc.tile_pool(name="fstage", bufs=2))
    wpool = ctx.enter_context(tc.tile_pool(name="w", bufs=KB))
    bcpool = ctx.enter_context(tc.tile_pool(name="bc", bufs=3))
    xpool = ctx.enter_context(tc.tile_pool(name="x", bufs=2))
    pspool = ctx.enter_context(tc.tile_pool(name="ps", bufs=2, space="PSUM"))
    opool = ctx.enter_context(tc.tile_pool(name="o", bufs=3))

    # ---- dequantize W: w = w4*s - z*s, one k-block (= one quant group) each.
    #      [s | z*s] comes pre-packed from the host; broadcast per block. ----
    w_tiles = []
    for q in range(KB // QCH):
        qw_t = qpool.tile([128, QCH * CB], U8)
        nc.gpsimd.dma_start(
            qw_t[:].rearrange("p (b c) -> p b c", b=QCH),
            qwd[q * QCH * 128 : (q + 1) * QCH * 128, :].rearrange(
                "(b p) c -> p b c", p=128
            ),
        )
        for i in range(QCH):
            b = q * QCH + i
            qw_b = qw_t[:, i * CB : (i + 1) * CB]
            bc = bcpool.tile([128, 2 * NS], F16)
            nc.gpsimd.dma_start(bc[:], comb[b : b + 1, :].partition_broadcast(128))
            wst = stpool.tile([128, NS], U8)
            nc.vector.tensor_scalar(wst[:, :CB], qw_b, 0xF, None, AOT.bitwise_and)
            nc.vector.tensor_scalar(
                wst[:, CB:], qw_b, 4, None, AOT.logical_shift_right
            )
            wstf = ftpool.tile([128, NS], F16)
            nc.gpsimd.tensor_copy(wstf[:], wst[:])
            w_t = wpool.tile([128, NS], F16)
            nc.vector.tensor_tensor(w_t[:], wstf[:], bc[:, :NS], AOT.mult)
            nc.vector.tensor_tensor(w_t[:], w_t[:], bc[:, NS:], AOT.subtract)
            w_tiles.append(w_t)

    # ---- matmul: stream pre-transposed x chunks, accumulate over K ----
    for c in range(NCH):
        xt_t = xpool.tile([128, KB * TC], F16, tag="xt")
        nc.gpsimd.dma_start(xt_t[:], xtd[c * 128 : (c + 1) * 128, :])
        for tau in range(TBLK):
            ps = pspool.tile([128, NS], F32)
            for b in range(KB):
                lhs = xt_t[:, b * TC + tau * 128 : b * TC + (tau + 1) * 128]
                for off, sz in SEGS:
                    nc.tensor.matmul(
                        ps[:, off : off + sz],
                        lhs,
                        w_tiles[b][:, off : off + sz],
                        start=(b == 0),
                        stop=(b == KB - 1),
                    )
            ob = opool.tile([128, NS], F16)
            nc.vector.tensor_copy(ob[:], ps[:])
            r0 = c * TC + tau * 128
            nc.gpsimd.dma_start(outd[r0 : r0 + 128, :], ob[:])


def build_kernel():
    nc = bacc.Bacc("TRN2", target_bir_lowering=False, debug=False)
    xtd = nc.dram_tensor("xt", [NCH * 128, KB * TC], F16, kind="ExternalInput").ap()
    qwd = nc.dram_tensor("qw", [K, CB], U8, kind="ExternalInput").ap()
    comb = nc.dram_tensor("comb", [G, 2 * NS], F16, kind="ExternalInput").ap()
    outd = nc.dram_tensor("out", [T, NS], F16, kind="ExternalOutput").ap()
    with tile.TileContext(nc) as tc, ExitStack() as ctx:
        _body(ctx, tc, xtd, qwd, comb, outd)
    nc.compile()
    return nc


_NC = None


def _get_nc():
    global _NC
    if _NC is None:
        _NC = build_kernel()
    return _NC


def _tile_xt(x):
    # x [T, K] -> xt [NCH*128, KB*TC] where
    # xt[c*128 + p, b*TC + t] = x[c*TC + t, b*128 + p]
    xt = np.ascontiguousarray(
        x.reshape(NCH, TC, KB, 128).transpose(0, 3, 2, 1)
    ).reshape(NCH * 128, KB * TC)
    return xt


def _perm_cols(a):
    # reference col n = c*8 + j -> device col: lo nibbles (j=2k) first, hi after
    # a [..., NS] -> [..., NS] with device order [c*4+k | CB + c*4+k]
    lead = a.shape[:-1]
    return np.ascontiguousarray(
        a.reshape(*lead, CS, 4, 2).transpose(*range(len(lead)), -1, -3, -2)
    ).reshape(*lead, NS)


def _unperm_out(o):
    # o [T, NS] device order -> reference column order
    return o.reshape(T, 2, CS, 4).transpose(0, 2, 3, 1).reshape(T, NS)


def make_in_maps(x, qweight, qzeros, scales):
    x = np.asarray(x, dtype=np.float16)
    qweight = np.asarray(qweight, dtype=np.int32)
    qzeros = np.asarray(qzeros, dtype=np.int32)
    scales = np.asarray(scales, dtype=np.float16)
    xt = _tile_xt(x)
    in_maps = []
    for c in range(NCORES):
        qw = np.ascontiguousarray(qweight[:, c * CS : (c + 1) * CS])
        qz = np.ascontiguousarray(qzeros[:, c * CS : (c + 1) * CS])
        sc = scales[:, c * NS : (c + 1) * NS]
        # group metadata in device (permuted) column order: [s | z*s]
        qz_u8 = qz.view(np.uint8).reshape(G, CB)
        z = np.concatenate([qz_u8 & 0xF, qz_u8 >> 4], axis=1).astype(np.float16)
        s_perm = _perm_cols(sc)
        comb = np.concatenate([s_perm, z * s_perm], axis=1)
        in_maps.append(
            {
                "xt": xt,
                "qw": qw.view(np.uint8).reshape(K, CB),
                "comb": comb,
            }
        )
    return in_maps


def run(in_maps, **kwargs):
    return run_bass_kernel_spmd(
        _get_nc(), in_maps, core_ids=list(range(NCORES)), **kwargs
    )


def assemble(res):
    outs = [_unperm_out(res.results[c]["out"]) for c in range(NCORES)]
    return np.concatenate(outs, axis=1)


def kernel(x, qweight, qzeros, scales):
    res = run(make_in_maps(x, qweight, qzeros, scales))
    return assemble(res)


# revision 12
# speedup vs baseline: 1.1194x; 1.1194x over previous
"""Trainium2 Bass kernel for int4-grouped-quantized linear (GPTQ-style).

out[8192, 11008] = x[8192, 4096] @ dequant(qweight, qzeros, scales)

Sharding: column-parallel over out_features N across 8 NeuronCores.

Device-side structure per core:
  - Dequantize the W shard [4096, 1376] on-chip: qweight is viewed as bytes
    (host-side reinterpret), so each nibble extraction is a single int ALU op
    and fuses with the scale multiply via scalar_tensor_tensor:
        w*s = (qw_u8 & 0xF) * s_bc   /   (qw_u8 >> 4) * s_bc
    then one tensor_tensor subtract of the precomputed broadcast zs = z*s.
    This orders W's columns [all lo-nibbles | all hi-nibbles]; the host
    un-permutes output columns (pure reshape/transpose).
  - Dense fp16 matmuls on the PE with fp32 PSUM accumulation; x is
    pre-transposed/tiled on the host (layout only) so the stationary
    [128 k, 128 t] tiles stream straight from DRAM, no on-device transposes.
"""

import sys

sys.path.insert(0, "/opt/trn_rl_repo")

from contextlib import ExitStack

import numpy as np

import concourse.bass as bass
from concourse import bacc
import concourse.tile as tile
from concourse import mybir
from concourse.bass_utils import run_bass_kernel_spmd

AOT = mybir.AluOpType
F16, I32, U8 = mybir.dt.float16, mybir.dt.int32, mybir.dt.uint8
F32 = mybir.dt.float32

T, K, N = 8192, 4096, 11008
NCORES = 8
NS = N // NCORES  # 1376 out cols per core
CS = NS // 8  # 172 packed int32 cols per core
CB = CS * 4  # 688 packed bytes per core (= NS/2)
G = 32  # quant groups (group size 128 == one k-block)
KB = K // 128  # 32 k-blocks
QCH = 8  # k-blocks per qweight load chunk
TC = 512  # t columns per x.T chunk
NCH = T // TC  # 16 chunks
TBLK = TC // 128  # 4 output row-blocks per chunk
SEGS = [(0, 512), (512, 512), (1024, 352)]  # N segments (PSUM bank sized)


def _body(ctx, tc, xtd, qwd, comb, outd):
    nc = tc.nc
    qpool = ctx.enter_context(tc.tile_pool(name="qwp", bufs=2))
    stpool = ctx.enter_context(tc.tile_pool(name="stage", bufs=2))
    wpool = ctx.enter_context(tc.tile_pool(name="w", bufs=KB))
    bcpool = ctx.enter_context(tc.tile_pool(name="bc", bufs=3))
    xpool = ctx.enter_context(tc.tile_pool(name="x", bufs=2))
    pspool = ctx.enter_context(tc.tile_pool(name="ps", bufs=2, space="PSUM"))
    opool = ctx.enter_context(tc.tile_pool(name="o", bufs=3))

    # ---- dequantize W: w = w4*s - z*s, one k-block (= one quant group) each.
    #      [s | z*s] comes pre-packed from the host; broadcast per block. ----
    w_tiles = []
    for q in range(KB // QCH):
        qw_t = qpool.tile([128, QCH * CB], U8)
        nc.gpsimd.dma_start(
            qw_t[:].rearrange("p (b c) -> p b c", b=QCH),
            qwd[q * QCH * 128 : (q + 1) * QCH * 128, :].rearrange(
                "(b p) c -> p b c", p=128
            ),
        )
        for i in range(QCH):
            b = q * QCH + i
            qw_b = qw_t[:, i * CB : (i + 1) * CB]
            bc = bcpool.tile([128, 2 * NS], F16)
            nc.gpsimd.dma_start(bc[:], comb[b : b + 1, :].partition_broadcast(128))
            wst = stpool.tile([128, NS], U8)
            nc.vector.tensor_scalar(wst[:, :CB], qw_b, 0xF, None, AOT.bitwise_and)
            nc.vector.tensor_scalar(
                wst[:, CB:], qw_b, 4, None, AOT.logical_shift_right
            )
            w_t = wpool.tile([128, NS], F16)
            nc.vector.tensor_tensor(w_t[:], bc[:, :NS], wst[:], AOT.mult)
            nc.vector.tensor_tensor(w_t[:], w_t[:], bc[:, NS:], AOT.subtract)
            w_tiles.append(w_t)

    # ---- matmul: stream pre-transposed x chunks, accumulate over K ----
    for c in range(NCH):
        xt_t = xpool.tile([128, KB * TC], F16, tag="xt")
        nc.gpsimd.dma_start(xt_t[:], xtd[c * 128 : (c + 1) * 128, :])
        for tau in range(TBLK):
            ps = pspool.tile([128, NS], F32)
            for b in range(KB):
                lhs = xt_t[:, b * TC + tau * 128 : b * TC + (tau + 1) * 128]
                for off, sz in SEGS:
                    nc.tensor.matmul(
                        ps[:, off : off + sz],
                        lhs,
                        w_tiles[b][:, off : off + sz],
                        start=(b == 0),
                        stop=(b == KB - 1),
                    )
            ob = opool.tile([128, NS], F16)
            nc.vector.tensor_copy(ob[:], ps[:])
            r0 = c * TC + tau * 128
            nc.gpsimd.dma_start(outd[r0 : r0 + 128, :], ob[:])


def build_kernel():
    nc = bacc.Bacc("TRN2", target_bir_lowering=False, debug=False)
    xtd = nc.dram_tensor("xt", [NCH * 128, KB * TC], F16, kind="ExternalInput").ap()
    qwd = nc.dram_tensor("qw", [K, CB], U8, kind="ExternalInput").ap()
    comb = nc.dram_tensor("comb", [G, 2 * NS], F16, kind="ExternalInput").ap()
    outd = nc.dram_tensor("out", [T, NS], F16, kind="ExternalOutput").ap()
    with tile.TileContext(nc) as tc, ExitStack() as ctx:
        _body(ctx, tc, xtd, qwd, comb, outd)
    nc.compile()
    return nc


_NC = None


def _get_nc():
    global _NC
    if _NC is None:
        _NC = build_kernel()
    return _NC


def _tile_xt(x):
    # x [T, K] -> xt [NCH*128, KB*TC] where
    # xt[c*128 + p, b*TC + t] = x[c*TC + t, b*128 + p]
    xt = np.ascontiguousarray(
        x.reshape(NCH, TC, KB, 128).transpose(0, 3, 2, 1)
    ).reshape(NCH * 128, KB * TC)
    return xt


def _perm_cols(a):
    # reference col n = c*8 + j -> device col: lo nibbles (j=2k) first, hi after
    # a [..., NS] -> [..., NS] with device order [c*4+k | CB + c*4+k]
    lead = a.shape[:-1]
    return np.ascontiguousarray(
        a.reshape(*lead, CS, 4, 2).transpose(*range(len(lead)), -1, -3, -2)
    ).reshape(*lead, NS)


def _unperm_out(o):
    # o [T, NS] device order -> reference column order
    return o.reshape(T, 2, CS, 4).transpose(0, 2, 3, 1).reshape(T, NS)


def make_in_maps(x, qweight, qzeros, scales):
    x = np.asarray(x, dtype=np.float16)
    qweight = np.asarray(qweight, dtype=np.int32)
    qzeros = np.asarray(qzeros, dtype=np.int32)
    scales = np.asarray(scales, dtype=np.float16)
    xt = _tile_xt(x)
    in_maps = []
    for c in range(NCORES):
        qw = np.ascontiguousarray(qweight[:, c * CS : (c + 1) * CS])
        qz = np.ascontiguousarray(qzeros[:, c * CS : (c + 1) * CS])
        sc = scales[:, c * NS : (c + 1) * NS]
        # group metadata in device (permuted) column order: [s | z*s]
        qz_u8 = qz.view(np.uint8).reshape(G, CB)
        z = np.concatenate([qz_u8 & 0xF, qz_u8 >> 4], axis=1).astype(np.float16)
        s_perm = _perm_cols(sc)
        comb = np.concatenate([s_perm, z * s_perm], axis=1)
        in_maps.append(
            {
                "xt": xt,
                "qw": qw.view(np.uint8).reshape(K, CB),
                "comb": comb,
            }
        )
    return in_maps


def run(in_maps, **kwargs):
    return run_bass_kernel_spmd(
        _get_nc(), in_maps, core_ids=list(range(NCORES)), **kwargs
    )


def assemble(res):
    outs = [_unperm_out(res.results[c]["out"]) for c in range(NCORES)]
    return np.concatenate(outs, axis=1)


def kernel(x, qweight, qzeros, scales):
    res = run(make_in_maps(x, qweight, qzeros, scales))
    return assemble(res)


# revision 13
# speedup vs baseline: 1.1334x; 1.0125x over previous
"""Trainium2 Bass kernel for int4-grouped-quantized linear (GPTQ-style).

out[8192, 11008] = x[8192, 4096] @ dequant(qweight, qzeros, scales)

Sharding: column-parallel over out_features N across 8 NeuronCores.

Device-side structure per core:
  - W dequant runs on DVE from a host byte-view of qweight: each nibble plane
    needs a single int ALU op (lo: &0xF, hi: >>4), then mult by s and subtract
    z*s using per-block [s | z*s] rows partition-broadcast via DMA. W's
    columns are ordered [all lo-nibbles | all hi-nibbles]; the host
    un-permutes output columns (pure reshape/transpose).
  - The GEMM runs in two N-half passes: pass A streams every output row-block
    against the lo-half of W (ready after only 32 lo-plane dequants), while
    the hi-half dequantizes in the background; pass B covers the hi-half.
    This halves the startup window in which the PE would starve on W.
  - x is pre-transposed/tiled on the host (layout only) into per-row-block
    [128 k, 128 t] stationary tiles; no on-device transposes. fp16 matmuls,
    fp32 PSUM accumulation.
"""

import sys

sys.path.insert(0, "/opt/trn_rl_repo")

from contextlib import ExitStack

import numpy as np

import concourse.bass as bass
from concourse import bacc
import concourse.tile as tile
from concourse import mybir
from concourse.bass_utils import run_bass_kernel_spmd

AOT = mybir.AluOpType
F16, I32, U8 = mybir.dt.float16, mybir.dt.int32, mybir.dt.uint8
F32 = mybir.dt.float32

T, K, N = 8192, 4096, 11008
NCORES = 8
NS = N // NCORES  # 1376 out cols per core
CS = NS // 8  # 172 packed int32 cols per core
CB = CS * 4  # 688 packed bytes per core (= NS/2, one nibble plane)
G = 32  # quant groups (group size 128 == one k-block)
KB = K // 128  # 32 k-blocks
QCH = 8  # k-blocks per qweight load chunk
NT = T // 128  # 64 output row-blocks
HSEGS = [(0, 512), (512, 176)]  # N-half segments (PSUM bank sized)


def _body(ctx, tc, xtd, qwd, comb, outd):
    nc = tc.nc
    qpool = ctx.enter_context(tc.tile_pool(name="qwp", bufs=2))
    stpool = ctx.enter_context(tc.tile_pool(name="stage", bufs=2))
    wpool = ctx.enter_context(tc.tile_pool(name="w", bufs=2 * KB))
    bcpool = ctx.enter_context(tc.tile_pool(name="bc", bufs=3))
    xpool = ctx.enter_context(tc.tile_pool(name="x", bufs=6))
    pspool = ctx.enter_context(tc.tile_pool(name="ps", bufs=2, space="PSUM"))
    opool = ctx.enter_context(tc.tile_pool(name="o", bufs=4))

    # ---- dequantize one nibble plane of W: w = w4*s - z*s per k-block.
    #      comb rows hold [sL | zsL | sR | zsR] in device column order. ----
    def dequant_half(h):
        w_tiles = []
        for q in range(KB // QCH):
            qw_t = qpool.tile([128, QCH * CB], U8, tag="qw")
            nc.gpsimd.dma_start(
                qw_t[:].rearrange("p (b c) -> p b c", b=QCH),
                qwd[q * QCH * 128 : (q + 1) * QCH * 128, :].rearrange(
                    "(b p) c -> p b c", p=128
                ),
            )
            for i in range(QCH):
                b = q * QCH + i
                qw_b = qw_t[:, i * CB : (i + 1) * CB]
                bc = bcpool.tile([128, 2 * CB], F16, tag="bc")
                nc.gpsimd.dma_start(
                    bc[:],
                    comb[b : b + 1, 2 * h * CB : 2 * (h + 1) * CB]
                    .partition_broadcast(128),
                )
                wst = stpool.tile([128, CB], U8, tag="wst")
                if h == 0:
                    nc.vector.tensor_scalar(
                        wst[:], qw_b, 0xF, None, AOT.bitwise_and
                    )
                else:
                    nc.vector.tensor_scalar(
                        wst[:], qw_b, 4, None, AOT.logical_shift_right
                    )
                w_t = wpool.tile([128, CB], F16)
                nc.vector.tensor_tensor(w_t[:], bc[:, :CB], wst[:], AOT.mult)
                nc.vector.tensor_tensor(w_t[:], w_t[:], bc[:, CB:], AOT.subtract)
                w_tiles.append(w_t)
        return w_tiles

    def gemm_half(w_tiles, h):
        for tg in range(NT):
            xt_t = xpool.tile([128, KB * 128], F16, tag="xt")
            nc.gpsimd.dma_start(xt_t[:], xtd[tg * 128 : (tg + 1) * 128, :])
            ps = pspool.tile([128, CB], F32)
            for b in range(KB):
                lhs = xt_t[:, b * 128 : (b + 1) * 128]
                for off, sz in HSEGS:
                    nc.tensor.matmul(
                        ps[:, off : off + sz],
                        lhs,
                        w_tiles[b][:, off : off + sz],
                        start=(b == 0),
                        stop=(b == KB - 1),
                    )
            ob = opool.tile([128, CB], F16)
            nc.vector.tensor_copy(ob[:], ps[:])
            r0 = tg * 128
            nc.gpsimd.dma_start(
                outd[r0 : r0 + 128, h * CB : (h + 1) * CB], ob[:]
            )

    wL = dequant_half(0)
    wR = dequant_half(1)
    gemm_half(wL, 0)
    gemm_half(wR, 1)


def build_kernel():
    nc = bacc.Bacc("TRN2", target_bir_lowering=False, debug=False)
    xtd = nc.dram_tensor("xt", [NT * 128, KB * 128], F16, kind="ExternalInput").ap()
    qwd = nc.dram_tensor("qw", [K, CB], U8, kind="ExternalInput").ap()
    comb = nc.dram_tensor("comb", [G, 4 * CB], F16, kind="ExternalInput").ap()
    outd = nc.dram_tensor("out", [T, NS], F16, kind="ExternalOutput").ap()
    with tile.TileContext(nc) as tc, ExitStack() as ctx:
        _body(ctx, tc, xtd, qwd, comb, outd)
    nc.compile()
    return nc


_NC = None


def _get_nc():
    global _NC
    if _NC is None:
        _NC = build_kernel()
    return _NC


def _tile_xt(x):
    # x [T, K] -> xt [NT*128, KB*128] where
    # xt[tg*128 + p, b*128 + t] = x[tg*128 + t, b*128 + p]
    return np.ascontiguousarray(
        x.reshape(NT, 128, KB, 128).transpose(0, 3, 2, 1)
    ).reshape(NT * 128, KB * 128)


def _perm_cols(a):
    # reference col n = c*8 + j -> device col: lo nibbles (j=2k) first, hi after
    lead = a.shape[:-1]
    return np.ascontiguousarray(
        a.reshape(*lead, CS, 4, 2).transpose(*range(len(lead)), -1, -3, -2)
    ).reshape(*lead, NS)


def _unperm_out(o):
    # o [T, NS] device order -> reference column order
    return o.reshape(T, 2, CS, 4).transpose(0, 2, 3, 1).reshape(T, NS)


def make_in_maps(x, qweight, qzeros, scales):
    x = np.asarray(x, dtype=np.float16)
    qweight = np.asarray(qweight, dtype=np.int32)
    qzeros = np.asarray(qzeros, dtype=np.int32)
    scales = np.asarray(scales, dtype=np.float16)
    xt = _tile_xt(x)
    in_maps = []
    for c in range(NCORES):
        qw = np.ascontiguousarray(qweight[:, c * CS : (c + 1) * CS])
        qz = np.ascontiguousarray(qzeros[:, c * CS : (c + 1) * CS])
        sc = scales[:, c * NS : (c + 1) * NS]
        # group metadata in device (permuted) column order: [sL, zsL, sR, zsR]
        qz_u8 = qz.view(np.uint8).reshape(G, CB)
        z = np.concatenate([qz_u8 & 0xF, qz_u8 >> 4], axis=1).astype(np.float16)
        s_perm = _perm_cols(sc)
        zs = z * s_perm
        comb = np.concatenate(
            [s_perm[:, :CB], zs[:, :CB], s_perm[:, CB:], zs[:, CB:]], axis=1
        )
        in_maps.append(
            {
                "xt": xt,
                "qw": qw.view(np.uint8).reshape(K, CB),
                "comb": comb,
            }
        )
    return in_maps


def run(in_maps, **kwargs):
    return run_bass_kernel_spmd(
        _get_nc(), in_maps, core_ids=list(range(NCORES)), **kwargs
    )


def assemble(res):
    outs = [_unperm_out(res.results[c]["out"]) for c in range(NCORES)]
    return np.concatenate(outs, axis=1)


def kernel(x, qweight, qzeros, scales):
    res = run(make_in_maps(x, qweight, qzeros, scales))
    return assemble(res)


# revision 15
# speedup vs baseline: 1.1406x; 1.0063x over previous
"""Trainium2 Bass kernel for int4-grouped-quantized linear (GPTQ-style).

out[8192, 11008] = x[8192, 4096] @ dequant(qweight, qzeros, scales)

Sharding: column-parallel over out_features N across 8 NeuronCores.

Device-side structure per core:
  - W dequant runs on DVE from a host byte-view of qweight: each nibble plane
    needs a single int ALU op (lo: &0xF, hi: >>4), then mult by s and subtract
    z*s using per-block [s | z*s] rows partition-broadcast via DMA. W's
    columns are ordered [all lo-nibbles | all hi-nibbles]; the host
    un-permutes output columns (pure reshape/transpose).
  - The GEMM runs in two N-half passes: pass A streams every output row-block
    against the lo-half of W (ready after only 32 lo-plane dequants), while
    the hi-half dequantizes in the background; pass B covers the hi-half.
    This halves the startup window in which the PE would starve on W.
  - x is pre-transposed/tiled on the host (layout only) into per-row-block
    [128 k, 128 t] stationary tiles; no on-device transposes. fp16 matmuls,
    fp32 PSUM accumulation.
"""

import sys

sys.path.insert(0, "/opt/trn_rl_repo")

from contextlib import ExitStack

import numpy as np

import concourse.bass as bass
from concourse import bacc
import concourse.tile as tile
from concourse import mybir
from concourse.bass_utils import run_bass_kernel_spmd

AOT = mybir.AluOpType
F16, I32, U8 = mybir.dt.float16, mybir.dt.int32, mybir.dt.uint8
F32 = mybir.dt.float32

T, K, N = 8192, 4096, 11008
NCORES = 8
NS = N // NCORES  # 1376 out cols per core
CS = NS // 8  # 172 packed int32 cols per core
CB = CS * 4  # 688 packed bytes per core (= NS/2, one nibble plane)
G = 32  # quant groups (group size 128 == one k-block)
KB = K // 128  # 32 k-blocks
QCH = 8  # k-blocks per qweight load chunk
NT = T // 128  # 64 output row-blocks
HSEGS = [(0, 512), (512, 176)]  # N-half segments (PSUM bank sized)


def _body(ctx, tc, xtd, qwd, comb, outd):
    nc = tc.nc
    qpool = ctx.enter_context(tc.tile_pool(name="qwp", bufs=2))
    stpool = ctx.enter_context(tc.tile_pool(name="stage", bufs=2))
    wpool = ctx.enter_context(tc.tile_pool(name="w", bufs=2 * KB))
    bcpool = ctx.enter_context(tc.tile_pool(name="bc", bufs=3))
    xpool = ctx.enter_context(tc.tile_pool(name="x", bufs=6))
    pspool = ctx.enter_context(tc.tile_pool(name="ps", bufs=3, space="PSUM"))
    opool = ctx.enter_context(tc.tile_pool(name="o", bufs=4))
    dpool = ctx.enter_context(tc.tile_pool(name="dummy", bufs=1))
    dpspool = ctx.enter_context(tc.tile_pool(name="dps", bufs=1, space="PSUM"))

    # ---- HAM warm-up: keep the PE busy from t=0 so the clock gate opens
    #      (K=8/8) before real matmuls start; results are discarded. ----
    dum = dpool.tile([128, 512], F16)
    nc.gpsimd.memset(dum[:], 0.0)
    dps = dpspool.tile([128, 512], F32)
    for _ in range(80):
        nc.tensor.matmul(dps[:], dum[:, :128], dum[:], start=True, stop=True)

    # ---- prefetch the first x.T tiles ahead of the dequant DMA burst ----
    xt_pre = []
    for tg in range(6):
        xt_t = xpool.tile([128, KB * 128], F16, tag="xt")
        nc.gpsimd.dma_start(xt_t[:], xtd[tg * 128 : (tg + 1) * 128, :])
        xt_pre.append(xt_t)

    # ---- dequantize one nibble plane of W: w = w4*s - z*s per k-block.
    #      comb rows hold [sL | zsL | sR | zsR] in device column order. ----
    def dequant_half(h):
        w_tiles = []
        for q in range(KB // QCH):
            qw_t = qpool.tile([128, QCH * CB], U8, tag="qw")
            nc.gpsimd.dma_start(
                qw_t[:].rearrange("p (b c) -> p b c", b=QCH),
                qwd[q * QCH * 128 : (q + 1) * QCH * 128, :].rearrange(
                    "(b p) c -> p b c", p=128
                ),
            )
            for i in range(QCH):
                b = q * QCH + i
                qw_b = qw_t[:, i * CB : (i + 1) * CB]
                bc = bcpool.tile([128, 2 * CB], F16, tag="bc")
                nc.gpsimd.dma_start(
                    bc[:],
                    comb[b : b + 1, 2 * h * CB : 2 * (h + 1) * CB]
                    .partition_broadcast(128),
                )
                wst = stpool.tile([128, CB], U8, tag="wst")
                if h == 0:
                    nc.vector.tensor_scalar(
                        wst[:], qw_b, 0xF, None, AOT.bitwise_and
                    )
                else:
                    nc.vector.tensor_scalar(
                        wst[:], qw_b, 4, None, AOT.logical_shift_right
                    )
                w_t = wpool.tile([128, CB], F16)
                nc.vector.tensor_tensor(w_t[:], bc[:, :CB], wst[:], AOT.mult)
                nc.vector.tensor_tensor(w_t[:], w_t[:], bc[:, CB:], AOT.subtract)
                w_tiles.append(w_t)
        return w_tiles

    def gemm_half(w_tiles, h):
        for tg in range(NT):
            if h == 0 and tg < len(xt_pre):
                xt_t = xt_pre[tg]
            else:
                xt_t = xpool.tile([128, KB * 128], F16, tag="xt")
                nc.gpsimd.dma_start(xt_t[:], xtd[tg * 128 : (tg + 1) * 128, :])
            ps = pspool.tile([128, CB], F32)
            for b in range(KB):
                lhs = xt_t[:, b * 128 : (b + 1) * 128]
                for off, sz in HSEGS:
                    nc.tensor.matmul(
                        ps[:, off : off + sz],
                        lhs,
                        w_tiles[b][:, off : off + sz],
                        start=(b == 0),
                        stop=(b == KB - 1),
                    )
            ob = opool.tile([128, CB], F16)
            nc.vector.tensor_copy(ob[:], ps[:])
            r0 = tg * 128
            nc.gpsimd.dma_start(
                outd[r0 : r0 + 128, h * CB : (h + 1) * CB], ob[:]
            )

    wL = dequant_half(0)
    wR = dequant_half(1)
    gemm_half(wL, 0)
    gemm_half(wR, 1)


def build_kernel():
    nc = bacc.Bacc("TRN2", target_bir_lowering=False, debug=False)
    xtd = nc.dram_tensor("xt", [NT * 128, KB * 128], F16, kind="ExternalInput").ap()
    qwd = nc.dram_tensor("qw", [K, CB], U8, kind="ExternalInput").ap()
    comb = nc.dram_tensor("comb", [G, 4 * CB], F16, kind="ExternalInput").ap()
    outd = nc.dram_tensor("out", [T, NS], F16, kind="ExternalOutput").ap()
    with tile.TileContext(nc) as tc, ExitStack() as ctx:
        _body(ctx, tc, xtd, qwd, comb, outd)
    nc.compile()
    return nc


_NC = None


def _get_nc():
    global _NC
    if _NC is None:
        _NC = build_kernel()
    return _NC


def _tile_xt(x):
    # x [T, K] -> xt [NT*128, KB*128] where
    # xt[tg*128 + p, b*128 + t] = x[tg*128 + t, b*128 + p]
    return np.ascontiguousarray(
        x.reshape(NT, 128, KB, 128).transpose(0, 3, 2, 1)
    ).reshape(NT * 128, KB * 128)


def _perm_cols(a):
    # reference col n = c*8 + j -> device col: lo nibbles (j=2k) first, hi after
    lead = a.shape[:-1]
    return np.ascontiguousarray(
        a.reshape(*lead, CS, 4, 2).transpose(*range(len(lead)), -1, -3, -2)
    ).reshape(*lead, NS)


def _unperm_out(o):
    # o [T, NS] device order -> reference column order
    return o.reshape(T, 2, CS, 4).transpose(0, 2, 3, 1).reshape(T, NS)


def make_in_maps(x, qweight, qzeros, scales):
    x = np.asarray(x, dtype=np.float16)
    qweight = np.asarray(qweight, dtype=np.int32)
    qzeros = np.asarray(qzeros, dtype=np.int32)
    scales = np.asarray(scales, dtype=np.float16)
    xt = _tile_xt(x)
    in_maps = []
    for c in range(NCORES):
        qw = np.ascontiguousarray(qweight[:, c * CS : (c + 1) * CS])
        qz = np.ascontiguousarray(qzeros[:, c * CS : (c + 1) * CS])
        sc = scales[:, c * NS : (c + 1) * NS]
        # group metadata in device (permuted) column order: [sL, zsL, sR, zsR]
        qz_u8 = qz.view(np.uint8).reshape(G, CB)
        z = np.concatenate([qz_u8 & 0xF, qz_u8 >> 4], axis=1).astype(np.float16)
        s_perm = _perm_cols(sc)
        zs = z * s_perm
        comb = np.concatenate(
            [s_perm[:, :CB], zs[:, :CB], s_perm[:, CB:], zs[:, CB:]], axis=1
        )
        in_maps.append(
            {
                "xt": xt,
                "qw": qw.view(np.uint8).reshape(K, CB),
                "comb": comb,
            }
        )
    return in_maps


def run(in_maps, **kwargs):
    return run_bass_kernel_spmd(
        _get_nc(), in_maps, core_ids=list(range(NCORES)), **kwargs
    )


def assemble(res):
    outs = [_unperm_out(res.results[c]["out"]) for c in range(NCORES)]
    return np.concatenate(outs, axis=1)


def kernel(x, qweight, qzeros, scales):
    res = run(make_in_maps(x, qweight, qzeros, scales))
    return assemble(res)
